# revision 11
# baseline (speedup 1.0000x reference)
"""DepAttention kernel for Trainium2 (Bass/Tile), data-parallel over batch.

score[b,i,j] = (<val[b,i],val[b,j]> + <dep[b,i,j],dep[b,j,i]>) / sqrt(D)
out = exp(score)*adj / (rowsum(exp(score)*adj) + 1e-10)

score is symmetric in (i,j), so each core (one batch element) computes
each unordered pair's dep dot product exactly once and mirrors the rest
with PE transposes. The host pre-gathers the dep operand streams into
dense fp16 tiles so that every DMA is a full-128-partition contiguous
read (DMA time is per-partition bytes): for each pair-group tile,
partition p's row holds A = dep[i,j,:] and (second tensor) B =
dep[j,i,:]. fp16 halves HBM traffic and doubles DVE multiply/add
throughput (packed 2x mode); the d-reduction runs as an in-place fp16
add tree (128->64->32->16) plus one f32 reduce_sum over 16.

Tiles per core (N=256 = 2 row blocks):
 - off-diag block (0,1): 4 col chunks of 32 from offA/offB; each chunk
   is val-added then PE-transposed into score1[32c:32c+32, 0:128]
   (val score is symmetric too), so the (1,0) mirror pipelines.
 - pair tiles c=1..3: block00's strict-upper col chunk c (partitions
   0..32c) packed with block11's strict-lower chunk c-1 (partitions
   32c..128). One reduce into a tmp, then ACT copies into the two
   score regions (partition windows must be 32-aligned).
 - interlocked minis: the 4 diagonal 32x32 squares of block0
   (upper+diag) and block1 (strict lower) share one [128,33,D] tile;
   slot 32 carries block1's diagonal self-pairs. A host mask applies
   upper/lower selection with 0.5 on block0's diagonal; block1's
   diagonal is injected as (identity * dvec) + score via
   scalar_tensor_tensor. The later transpose-mirror then doubles the
   halved diagonals and fills all mirrored entries exactly once.
"""

import numpy as np

import concourse.bacc as bacc
import concourse.tile as tile
import concourse.mybir as mybir
from concourse.bass_utils import run_bass_kernel_spmd

B, N, D = 8, 256, 128
W = 32        # columns per chunk
NCH = 128 // W
SCALE = 1.0 / np.sqrt(np.float32(D))
EPS = 1e-10
F32 = mybir.dt.float32
F16 = mybir.dt.float16

_NC = None


def build_nc(reps=1):
    """reps>1 unrolls the whole computation (for paired-slope timing)."""
    nc = bacc.Bacc("TRN2", target_bir_lowering=False, debug=False, num_devices=8)

    offA = nc.dram_tensor("offA", [128, 128, D], F16, kind="ExternalInput")
    offB = nc.dram_tensor("offB", [128, 128, D], F16, kind="ExternalInput")
    pkA = nc.dram_tensor("pkA", [3, 128, W, D], F16, kind="ExternalInput")
    pkB = nc.dram_tensor("pkB", [3, 128, W, D], F16, kind="ExternalInput")
    mnA = nc.dram_tensor("mnA", [128, W + 1, D], F16, kind="ExternalInput")
    mnB = nc.dram_tensor("mnB", [128, W + 1, D], F16, kind="ExternalInput")
    valT = nc.dram_tensor("valT", [D, N], F32, kind="ExternalInput")
    ladj = nc.dram_tensor("ladj", [N, N], F32, kind="ExternalInput")
    ident = nc.dram_tensor("ident", [128, 128], F32, kind="ExternalInput")
    mmask = nc.dram_tensor("mmask", [128, 2 * W], F32, kind="ExternalInput")
    out = nc.dram_tensor("out", [N, N], F32, kind="ExternalOutput")

    with tile.TileContext(nc) as tc:
        with (
            tc.tile_pool(name="a", bufs=4) as a_pool,
            tc.tile_pool(name="b", bufs=4) as b_pool,
            tc.tile_pool(name="persist", bufs=1) as pp,
            tc.tile_pool(name="psum", bufs=1, space="PSUM") as psp,
        ):
            # persistent tiles
            vt = pp.tile([D, N], F32, tag="vt")
            id_t = pp.tile([128, 128], F32, tag="id")
            adj_t = [
                pp.tile([128, N], F32, tag=f"adj{i}", name=f"adj{i}") for i in range(2)
            ]
            mm_t = pp.tile([128, 2 * W], F32, tag="mm")
            scratch = pp.tile([128, 1], F32, tag="scratch")

            nc.gpsimd.dma_start(vt[:], valT[:])
            nc.gpsimd.dma_start(id_t[:], ident[:])
            nc.gpsimd.dma_start(mm_t[:], mmask[:])
            for i in range(2):
                nc.gpsimd.dma_start(adj_t[i][:], ladj[128 * i : 128 * (i + 1), :])
            # prime the ACT exp table before the epilogue needs it
            nc.vector.memset(scratch[:], 0.0)
            nc.scalar.activation(
                scratch[:], scratch[:], mybir.ActivationFunctionType.Exp, scale=1.0
            )

            def pwin(lo, hi):
                """Split [lo,hi) into HW-legal partition windows
                (starts 0/32/64/96; span 64 only from 0/64, 128 from 0)."""
                res = []
                while lo < hi:
                    if lo == 0 and hi >= 128:
                        w = 128
                    elif lo % 64 == 0 and hi - lo >= 64:
                        w = 64
                    else:
                        w = min(32, hi - lo)
                    res.append((lo, lo + w))
                    lo += w
                return res

            def chain_and_reduce(a_t, b_t, red_out, gp_add1=False, gp_add4=False):
                """In-place fp16 add tree over d then one f32 reduce."""
                nc.vector.tensor_mul(a_t[:], a_t[:], b_t[:])
                eng1 = nc.gpsimd if gp_add1 else nc.vector
                eng1.tensor_add(
                    a_t[:, :, 0:64], a_t[:, :, 0:64], a_t[:, :, 64:128]
                )
                nc.vector.tensor_add(
                    a_t[:, :, 0:32], a_t[:, :, 0:32], a_t[:, :, 32:64]
                )
                nc.vector.tensor_add(
                    a_t[:, :, 0:16], a_t[:, :, 0:16], a_t[:, :, 16:32]
                )
                if gp_add4:
                    nc.gpsimd.tensor_add(
                        a_t[:, :, 0:8], a_t[:, :, 0:8], a_t[:, :, 8:16]
                    )
                    nc.vector.reduce_sum(
                        red_out, a_t[:, :, 0:8], axis=mybir.AxisListType.X
                    )
                else:
                    nc.vector.reduce_sum(
                        red_out, a_t[:, :, 0:16], axis=mybir.AxisListType.X
                    )

            for _rep in range(reps):
                score = [
                    pp.tile([128, N], F32, tag=f"score{i}", name=f"score{i}", bufs=2)
                    for i in range(2)
                ]
                expv = [
                    pp.tile([128, N], F32, tag=f"expv{i}", name=f"expv{i}", bufs=2)
                    for i in range(2)
                ]
                mtmp = pp.tile([128, W + 1], F32, tag="mtmp", name="mtmp", bufs=2)
                mU = pp.tile([128, W], F32, tag="mU", name="mU", bufs=2)
                mL = pp.tile([128, W], F32, tag="mL", name="mL", bufs=2)
                dvec = pp.tile([128, 1], F32, tag="dvec", name="dvec", bufs=2)
                ptile = [
                    pp.tile([128, W], F32, tag=f"ptl{c}", name=f"ptl{c}", bufs=2)
                    for c in range(1, NCH)
                ]
                den = [
                    pp.tile([128, 1], F32, tag=f"den{i}", name=f"den{i}", bufs=2)
                    for i in range(2)
                ]
                denb = [
                    pp.tile([128, 1], F32, tag=f"denb{i}", name=f"denb{i}", bufs=2)
                    for i in range(2)
                ]
                rec = [
                    pp.tile([128, 1], F32, tag=f"rec{i}", name=f"rec{i}", bufs=2)
                    for i in range(2)
                ]
                psum_sv = [
                    psp.tile([128, N], F32, tag=f"sv{i}", name=f"sv{i}", bufs=1)
                    for i in range(2)
                ]

                # zero regions the mirror-add will read where uncomputed
                nc.gpsimd.memset(score[0][:, 0:128], 0.0)
                nc.gpsimd.memset(score[1][:, 128:256], 0.0)

                # val part: score_val[I] = valT[:, I*128:+128].T @ valT
                for i in range(2):
                    nc.tensor.matmul(
                        psum_sv[i][:],
                        vt[:, 128 * i : 128 * (i + 1)],
                        vt[:],
                        start=True,
                        stop=True,
                    )

                def off_chunk(j0, w, gp=False):
                    jr = j0 - 128
                    tg = "a_t" if w == W else "a_s"
                    a_t = a_pool.tile([128, w, D], F16, name="a_t", tag=tg)
                    b_t = b_pool.tile([128, w, D], F16, name="b_t", tag="b" + tg[1:])
                    nc.sync.dma_start(a_t[:], offA[:, jr : jr + w, :])
                    nc.scalar.dma_start(b_t[:], offB[:, jr : jr + w, :])
                    chain_and_reduce(a_t, b_t, score[0][:, j0 : j0 + w], gp_add1=gp)

                def pair_chunk(c):
                    p = W * c
                    cols0 = (W * c, W * c + W)
                    cols1 = (128 + W * (c - 1), 128 + W * c)
                    a_t = a_pool.tile([128, W, D], F16, name="a_t", tag="a_t")
                    b_t = b_pool.tile([128, W, D], F16, name="b_t", tag="b_t")
                    nc.sync.dma_start(a_t[:], pkA[c - 1])
                    nc.scalar.dma_start(b_t[:], pkB[c - 1])
                    pt = ptile[c - 1]
                    chain_and_reduce(a_t, b_t, pt[:], gp_add1=True, gp_add4=True)
                    for lo, hi in pwin(0, p):
                        nc.scalar.copy(
                            score[0][lo:hi, cols0[0] : cols0[1]], pt[lo:hi, :]
                        )
                    for lo, hi in pwin(p, 128):
                        nc.scalar.copy(
                            score[1][lo:hi, cols1[0] : cols1[1]], pt[lo:hi, :]
                        )

                def mini_tile():
                    a_t = a_pool.tile([128, W + 1, D], F16, name="a_m", tag="a_m")
                    b_t = b_pool.tile([128, W + 1, D], F16, name="b_m", tag="b_m")
                    nc.sync.dma_start(a_t[:], mnA[:])
                    nc.scalar.dma_start(b_t[:], mnB[:])
                    chain_and_reduce(a_t, b_t, mtmp[:], gp_add1=True, gp_add4=True)
                    # upper(0.5-diag) / strict-lower mask selection
                    nc.vector.tensor_mul(mU[:], mtmp[:, 0:W], mm_t[:, 0:W])
                    nc.vector.tensor_mul(mL[:], mtmp[:, 0:W], mm_t[:, W : 2 * W])
                    nc.vector.tensor_scalar_mul(dvec[:], mtmp[:, W : W + 1], 0.5)
                    for m in range(NCH):
                        lo, hi = W * m, W * m + W
                        nc.scalar.copy(score[0][lo:hi, lo:hi], mU[lo:hi, :])
                        nc.scalar.copy(
                            score[1][lo:hi, 128 + lo : 128 + hi], mL[lo:hi, :]
                        )
                    # block1 diagonal: score1[:,128:256] += ident * dvec
                    nc.vector.scalar_tensor_tensor(
                        score[1][:, 128:256],
                        id_t[:],
                        dvec[:, 0:1],
                        score[1][:, 128:256],
                        op0=mybir.AluOpType.mult,
                        op1=mybir.AluOpType.add,
                    )

                # order: quick-ramp halves of off-chunk 0, diag work, rest
                off_chunk(128, 8)
                off_chunk(136, 8)
                off_chunk(144, 16)
                for c in range(1, NCH):
                    pair_chunk(c)
                mini_tile()

                psum_tr = [
                    psp.tile([32, 128], F32, tag=f"tr{c}", name=f"tr{c}", bufs=1)
                    for c in range(NCH)
                ]

                def mirror_off(c):
                    j0 = 128 + W * c
                    nc.vector.tensor_add(
                        score[0][:, j0 : j0 + W],
                        score[0][:, j0 : j0 + W],
                        psum_sv[0][:, j0 : j0 + W],
                    )
                    nc.tensor.transpose(
                        psum_tr[c][:], score[0][:, j0 : j0 + W], id_t[:]
                    )
                    nc.scalar.copy(
                        score[1][W * c : W * c + W, 0:128], psum_tr[c][:]
                    )

                mirror_off(0)
                for c in range(1, NCH):
                    off_chunk(128 + W * c, W, gp=(c < NCH - 1))
                    mirror_off(c)

                # ---- diagonal mirrors (dep-only, pre-val) ----
                psum_t = [
                    psp.tile([128, 128], F32, tag=f"mt{i}", name=f"mt{i}", bufs=1)
                    for i in range(2)
                ]
                nc.tensor.transpose(psum_t[0][:], score[0][:, 0:128], id_t[:])
                nc.vector.tensor_add(
                    score[0][:, 0:128], score[0][:, 0:128], psum_t[0][:]
                )
                nc.tensor.transpose(psum_t[1][:], score[1][:, 128:256], id_t[:])
                nc.vector.tensor_add(
                    score[1][:, 128:256], score[1][:, 128:256], psum_t[1][:]
                )

                # ---- val adds for the diagonal halves ----
                nc.vector.tensor_add(
                    score[0][:, 0:128], score[0][:, 0:128], psum_sv[0][:, 0:128]
                )
                nc.vector.tensor_add(
                    score[1][:, 128:256],
                    score[1][:, 128:256],
                    psum_sv[1][:, 128:256],
                )

                # ---- epilogue, split by halves for tail overlap ----
                for i in range(2):
                    for k, (lo, hi) in enumerate(((128, 256), (0, 128))):
                        # score*scale + logmask, then exp with den accumulation
                        nc.vector.scalar_tensor_tensor(
                            score[i][:, lo:hi],
                            score[i][:, lo:hi],
                            float(SCALE),
                            adj_t[i][:, lo:hi],
                            op0=mybir.AluOpType.mult,
                            op1=mybir.AluOpType.add,
                        )
                        nc.scalar.activation(
                            expv[i][:, lo:hi],
                            score[i][:, lo:hi],
                            mybir.ActivationFunctionType.Exp,
                            accum_out=(den[i] if k == 0 else denb[i])[:],
                        )
                    nc.vector.scalar_tensor_tensor(
                        den[i][:],
                        den[i][:],
                        float(EPS),
                        denb[i][:],
                        op0=mybir.AluOpType.add,
                        op1=mybir.AluOpType.add,
                    )
                    nc.vector.reciprocal(rec[i][:], den[i][:])
                    nc.vector.tensor_scalar_mul(expv[i][:], expv[i][:], rec[i][:, 0:1])
                    nc.sync.dma_start(out[128 * i : 128 * (i + 1), :], expv[i][:])

    nc.compile()
    return nc


def _pack_indices():
    """Row/col gather indices for the packed pair tiles [3,128,W] and the
    interlocked mini tile [128,W+1]. B-side swaps rows/cols."""
    p = np.arange(128)
    w = np.arange(W)
    prow = np.zeros((3, 128, W), np.int64)
    pcol = np.zeros((3, 128, W), np.int64)
    for c in range(1, NCH):
        pc = W * c
        prow[c - 1] = np.broadcast_to(
            np.where(p < pc, p, 128 + p)[:, None], (128, W)
        )
        pcol[c - 1] = np.where(
            p[:, None] < pc, W * c + w[None, :], 128 + W * (c - 1) + w[None, :]
        )
    r = p % W
    m = p // W
    mrow = np.zeros((128, W + 1), np.int64)
    mcol = np.zeros((128, W + 1), np.int64)
    up = w[None, :] >= r[:, None]
    mrow[:, 0:W] = np.where(up, (W * m + r)[:, None], (128 + W * m + r)[:, None])
    mcol[:, 0:W] = np.where(
        up, W * m[:, None] + w[None, :], 128 + W * m[:, None] + w[None, :]
    )
    mrow[:, W] = 128 + p
    mcol[:, W] = 128 + p
    return prow, pcol, mrow, mcol


_PROW, _PCOL, _MROW, _MCOL = _pack_indices()


def _make_mmask():
    p = np.arange(128)
    w = np.arange(W)
    r = (p % W)[:, None]
    mm = np.zeros((128, 2 * W), np.float32)
    mm[:, 0:W] = np.where(w[None, :] > r, 1.0, np.where(w[None, :] == r, 0.5, 0.0))
    mm[:, W : 2 * W] = np.where(w[None, :] < r, 1.0, 0.0)
    return mm


_MMASK = _make_mmask()


def make_in_map(val_out_b, dep_b, adj_b):
    """Per-core input dict from one batch element's full-precision inputs."""
    dep16 = np.asarray(dep_b).astype(np.float16)
    return {
        "offA": np.ascontiguousarray(dep16[0:128, 128:256, :]),
        "offB": np.ascontiguousarray(dep16[128:256, 0:128, :].transpose(1, 0, 2)),
        "pkA": np.ascontiguousarray(dep16[_PROW, _PCOL, :]),
        "pkB": np.ascontiguousarray(dep16[_PCOL, _PROW, :]),
        "mnA": np.ascontiguousarray(dep16[_MROW, _MCOL, :]),
        "mnB": np.ascontiguousarray(dep16[_MCOL, _MROW, :]),
        "valT": np.ascontiguousarray(val_out_b.T).astype(np.float32),
        "ladj": np.where(np.asarray(adj_b) > 0, np.float32(0), np.float32(-1e30)),
        "ident": np.eye(128, dtype=np.float32),
        "mmask": _MMASK,
    }


def _get_nc():
    global _NC
    if _NC is None:
        _NC = build_nc()
    return _NC


def kernel(val_out, dep_embed, adj):
    val_out = np.asarray(val_out, dtype=np.float32)
    dep_embed = np.asarray(dep_embed, dtype=np.float32)
    adj = np.asarray(adj, dtype=np.float32)
    assert val_out.shape == (B, N, D)
    assert dep_embed.shape == (B, N, N, D)
    assert adj.shape == (B, N, N)

    nc = _get_nc()
    in_maps = [make_in_map(val_out[b], dep_embed[b], adj[b]) for b in range(B)]
    res = run_bass_kernel_spmd(nc, in_maps, core_ids=list(range(B)))
    return np.stack([r["out"] for r in res.results])
